# revision 1
# baseline (speedup 1.0000x reference)
"""AEVComputer (ANI-style atomic environment vectors) on 8 NeuronCores.

Pure data-parallel: the 128-molecule batch is sharded 16-per-core across the
8 trn2 NeuronCores; all small parameter tensors are replicated. Output is
gathered back to the full (128, 32, 384) array.

Self-contained: shapes/sharding hardcoded for B=128, N=32, 8 cores.
"""
import numpy as np
import jax
import jax.numpy as jnp

RCR = 5.2
RCA = 3.5
NUM_SPECIES = 4
B, N = 128, 32
NCORES = 8


def _triu_index(ns):
    i, j = np.triu_indices(ns)
    t = np.zeros((ns, ns), dtype=np.int32)
    idx = np.arange(len(i), dtype=np.int32)
    t[i, j] = idx
    t[j, i] = idx
    return t


_TRIU = _triu_index(NUM_SPECIES)


def _aev_shard(coordinates, species, EtaR, ShfR, EtaA, Zeta, ShfA, ShfZ):
    """AEV for one shard: coordinates (b,N,3), species (b,N) -> (b,N,384)."""
    b, n, _ = coordinates.shape
    valid = species >= 0
    sp = jnp.clip(species, 0, NUM_SPECIES - 1)
    vec = coordinates[:, None, :, :] - coordinates[:, :, None, :]  # (b,N,N,3)
    d2 = jnp.sum(vec * vec, axis=-1)
    eye = jnp.eye(n, dtype=bool)
    d = jnp.sqrt(jnp.where(d2 > 0, d2, 1.0))
    pair_ok = valid[:, :, None] & valid[:, None, :] & (~eye)[None]

    # radial
    mR = pair_ok & (d <= RCR)
    fcR = 0.5 * jnp.cos(d * (np.pi / RCR)) + 0.5
    rt = 0.25 * jnp.exp(-EtaR[:, None] * (d[..., None, None] - ShfR[None, :]) ** 2) \
        * fcR[..., None, None]
    rt = rt * mR[..., None, None].astype(rt.dtype)
    rt = rt.reshape(b, n, n, -1)  # (b,N,N,16)
    oh = jax.nn.one_hot(species, NUM_SPECIES, dtype=rt.dtype)  # (b,N,4)
    radial = jnp.einsum('bijr,bjs->bisr', rt, oh).reshape(b, n, -1)  # (b,N,64)

    # angular — arccos-free: with x = 0.95*cos(theta) the reference's
    # cos(arccos(x) - ShfZ) equals x*cos(ShfZ) + sqrt(1-x^2)*sin(ShfZ)
    # (valid since arccos(x) is in [0, pi] where sin >= 0).
    mA = pair_ok & (d <= RCA)
    tmask = mA[:, :, :, None] & mA[:, :, None, :] & (~eye)[None, None, :, :]
    dij = d[:, :, :, None]
    dik = d[:, :, None, :]
    cosang = jnp.einsum('bijd,bikd->bijk', vec, vec) / jnp.maximum(dij * dik, 1e-10)
    x = 0.95 * cosang
    sx = jnp.sqrt(jnp.maximum(1.0 - x * x, 0.0))
    cosdelta = x[..., None] * jnp.cos(ShfZ) + sx[..., None] * jnp.sin(ShfZ)  # (b,N,N,N,8)
    f1 = ((1.0 + cosdelta) * 0.5) ** Zeta[0]  # (b,N,N,N,8)
    fcA = 0.5 * jnp.cos(d * (np.pi / RCA)) + 0.5
    fprod = fcA[:, :, :, None] * fcA[:, :, None, :]
    dmean = (dij + dik) * 0.5
    f2 = jnp.exp(-EtaA[0] * (dmean[..., None] - ShfA) ** 2)  # (b,N,N,N,4)
    w = 2.0 * fprod * tmask.astype(f1.dtype)  # (b,N,N,N)
    # term (b,N,N,N,4,8) -> flatten (ShfA, ShfZ) in that order = reference's
    # (EtaA, Zeta, ShfA, ShfZ) flattening with singleton EtaA/Zeta.
    term = w[..., None, None] * f2[..., :, None] * f1[..., None, :]
    term = term.reshape(b, n, n, n, 32)
    npairs = NUM_SPECIES * (NUM_SPECIES + 1) // 2
    pidx = jnp.asarray(_TRIU)[sp[:, :, None], sp[:, None, :]]  # (b,N,N)
    poh = jax.nn.one_hot(pidx, npairs, dtype=term.dtype)  # (b,N,N,10)
    angular = 0.5 * jnp.einsum('bijkf,bjkp->bipf', term, poh)
    angular = angular.reshape(b, n, -1)  # (b,N,320)

    return jnp.concatenate([radial, angular], axis=-1)  # (b,N,384)


_COMPILED = {}


def _get_fn(EtaR, ShfR, EtaA, Zeta, ShfA, ShfZ):
    key = (float(EtaR[0]), tuple(np.asarray(ShfR).tolist()), float(EtaA[0]),
           float(Zeta[0]), tuple(np.asarray(ShfA).tolist()),
           tuple(np.asarray(ShfZ).tolist()))
    if key not in _COMPILED:
        cEtaR = jnp.asarray(EtaR, jnp.float32)
        cShfR = jnp.asarray(ShfR, jnp.float32)
        cEtaA = jnp.asarray(EtaA, jnp.float32)
        cZeta = jnp.asarray(Zeta, jnp.float32)
        cShfA = jnp.asarray(ShfA, jnp.float32)
        cShfZ = jnp.asarray(ShfZ, jnp.float32)

        def shard_fn(coords, spec):
            return _aev_shard(coords, spec, cEtaR, cShfR, cEtaA, cZeta,
                              cShfA, cShfZ)

        _COMPILED[key] = jax.pmap(shard_fn, devices=jax.devices()[:NCORES])
    return _COMPILED[key]


def kernel(coordinates, EtaR, ShfR, EtaA, Zeta, ShfA, ShfZ, species):
    coords = np.ascontiguousarray(np.asarray(coordinates, np.float32)) \
        .reshape(NCORES, B // NCORES, N, 3)
    spec = np.ascontiguousarray(np.asarray(species, np.int32)) \
        .reshape(NCORES, B // NCORES, N)
    fn = _get_fn(np.asarray(EtaR), np.asarray(ShfR), np.asarray(EtaA),
                 np.asarray(Zeta), np.asarray(ShfA), np.asarray(ShfZ))
    out = fn(coords, spec)  # (8, 16, N, 384)
    return np.asarray(out, np.float32).reshape(B, N, 384)


# revision 3
# speedup vs baseline: 2.6051x; 2.6051x over previous
"""AEVComputer (ANI-style atomic environment vectors) on 8 NeuronCores.

Pure data-parallel: the 128-molecule batch is sharded 16-per-core across the
8 trn2 NeuronCores; all small parameter tensors are replicated. Output is
gathered back to the full (128, 32, 384) array.

Self-contained: shapes/sharding hardcoded for B=128, N=32, 8 cores.
"""
import numpy as np
import jax
import jax.numpy as jnp

RCR = 5.2
RCA = 3.5
NUM_SPECIES = 4
B, N = 128, 32
NCORES = 8


def _triu_index(ns):
    i, j = np.triu_indices(ns)
    t = np.zeros((ns, ns), dtype=np.int32)
    idx = np.arange(len(i), dtype=np.int32)
    t[i, j] = idx
    t[j, i] = idx
    return t


_TRIU = _triu_index(NUM_SPECIES)


def _aev_shard(coordinates, species, EtaR, ShfR, EtaA, Zeta, ShfA, ShfZ):
    """AEV for one shard: coordinates (b,N,3), species (b,N) -> (b,N,384)."""
    b, n, _ = coordinates.shape
    valid = species >= 0
    sp = jnp.clip(species, 0, NUM_SPECIES - 1)
    vec = coordinates[:, None, :, :] - coordinates[:, :, None, :]  # (b,N,N,3)
    d2 = jnp.sum(vec * vec, axis=-1)
    eye = jnp.eye(n, dtype=bool)
    d = jnp.sqrt(jnp.where(d2 > 0, d2, 1.0))
    pair_ok = valid[:, :, None] & valid[:, None, :] & (~eye)[None]

    # radial
    mR = pair_ok & (d <= RCR)
    fcR = 0.5 * jnp.cos(d * (np.pi / RCR)) + 0.5
    rt = 0.25 * jnp.exp(-EtaR[:, None] * (d[..., None, None] - ShfR[None, :]) ** 2) \
        * fcR[..., None, None]
    rt = rt * mR[..., None, None].astype(rt.dtype)
    rt = rt.reshape(b, n, n, -1)  # (b,N,N,16)
    oh = jax.nn.one_hot(species, NUM_SPECIES, dtype=rt.dtype)  # (b,N,4)
    radial = jnp.einsum('bijr,bjs->bisr', rt, oh).reshape(b, n, -1)  # (b,N,64)

    # angular — arccos-free: with x = 0.95*cos(theta) the reference's
    # cos(arccos(x) - ShfZ) equals x*cos(ShfZ) + sqrt(1-x^2)*sin(ShfZ)
    # (valid since arccos(x) is in [0, pi] where sin >= 0).
    mA = pair_ok & (d <= RCA)
    tmask = mA[:, :, :, None] & mA[:, :, None, :] & (~eye)[None, None, :, :]
    dij = d[:, :, :, None]
    dik = d[:, :, None, :]
    cosang = jnp.einsum('bijd,bikd->bijk', vec, vec) / jnp.maximum(dij * dik, 1e-10)
    x = 0.95 * cosang
    sx = jnp.sqrt(jnp.maximum(1.0 - x * x, 0.0))
    cosdelta = x[..., None] * jnp.cos(ShfZ) + sx[..., None] * jnp.sin(ShfZ)  # (b,N,N,N,8)
    base = (1.0 + cosdelta) * 0.5
    zeta = float(Zeta[0])
    if zeta == int(zeta) and 1 <= zeta <= 64:
        # integer power via square-and-multiply (avoids exp/log lowering)
        e = int(zeta)
        acc = None
        sq = base
        while e:
            if e & 1:
                acc = sq if acc is None else acc * sq
            e >>= 1
            if e:
                sq = sq * sq
        f1 = acc
    else:
        f1 = base ** zeta
    fcA = 0.5 * jnp.cos(d * (np.pi / RCA)) + 0.5
    fprod = fcA[:, :, :, None] * fcA[:, :, None, :]
    dmean = (dij + dik) * 0.5
    f2 = jnp.exp(-EtaA[0] * (dmean[..., None] - ShfA) ** 2)  # (b,N,N,N,4)
    w = 2.0 * fprod * tmask.astype(f1.dtype)  # (b,N,N,N)
    # term (b,N,N,N,4,8) -> flatten (ShfA, ShfZ) in that order = reference's
    # (EtaA, Zeta, ShfA, ShfZ) flattening with singleton EtaA/Zeta.
    term = w[..., None, None] * f2[..., :, None] * f1[..., None, :]
    term = term.reshape(b, n, n, n, 32)
    npairs = NUM_SPECIES * (NUM_SPECIES + 1) // 2
    pidx = jnp.asarray(_TRIU)[sp[:, :, None], sp[:, None, :]]  # (b,N,N)
    poh = jax.nn.one_hot(pidx, npairs, dtype=term.dtype)  # (b,N,N,10)
    angular = 0.5 * jnp.einsum('bijkf,bjkp->bipf', term, poh)
    angular = angular.reshape(b, n, -1)  # (b,N,320)

    return jnp.concatenate([radial, angular], axis=-1)  # (b,N,384)


_COMPILED = {}


def _get_fn(EtaR, ShfR, EtaA, Zeta, ShfA, ShfZ):
    key = (float(EtaR[0]), tuple(np.asarray(ShfR).tolist()), float(EtaA[0]),
           float(Zeta[0]), tuple(np.asarray(ShfA).tolist()),
           tuple(np.asarray(ShfZ).tolist()))
    if key not in _COMPILED:
        cEtaR = np.asarray(EtaR, np.float32)
        cShfR = np.asarray(ShfR, np.float32)
        cEtaA = np.asarray(EtaA, np.float32)
        cZeta = np.asarray(Zeta, np.float32)
        cShfA = np.asarray(ShfA, np.float32)
        cShfZ = np.asarray(ShfZ, np.float32)

        def shard_fn(coords, spec):
            return _aev_shard(coords, spec, cEtaR, cShfR, cEtaA, cZeta,
                              cShfA, cShfZ)

        _COMPILED[key] = jax.pmap(shard_fn, devices=jax.devices()[:NCORES])
    return _COMPILED[key]


def kernel(coordinates, EtaR, ShfR, EtaA, Zeta, ShfA, ShfZ, species):
    coords = np.ascontiguousarray(np.asarray(coordinates, np.float32)) \
        .reshape(NCORES, B // NCORES, N, 3)
    spec = np.ascontiguousarray(np.asarray(species, np.int32)) \
        .reshape(NCORES, B // NCORES, N)
    fn = _get_fn(np.asarray(EtaR), np.asarray(ShfR), np.asarray(EtaA),
                 np.asarray(Zeta), np.asarray(ShfA), np.asarray(ShfZ))
    out = fn(coords, spec)  # (8, 16, N, 384)
    return np.asarray(out, np.float32).reshape(B, N, 384)


# revision 4
# speedup vs baseline: 3.4579x; 1.3273x over previous
"""AEVComputer (ANI-style atomic environment vectors) on 8 NeuronCores.

Pure data-parallel: the 128-molecule batch is sharded 16-per-core across the
8 trn2 NeuronCores; all small parameter tensors are replicated. Output is
gathered back to the full (128, 32, 384) array.

Self-contained: shapes/sharding hardcoded for B=128, N=32, 8 cores.
"""
import numpy as np
import jax
import jax.numpy as jnp

RCR = 5.2
RCA = 3.5
NUM_SPECIES = 4
B, N = 128, 32
NCORES = 8


def _triu_index(ns):
    i, j = np.triu_indices(ns)
    t = np.zeros((ns, ns), dtype=np.int32)
    idx = np.arange(len(i), dtype=np.int32)
    t[i, j] = idx
    t[j, i] = idx
    return t


_TRIU = _triu_index(NUM_SPECIES)


def _aev_shard(coordinates, species, EtaR, ShfR, EtaA, Zeta, ShfA, ShfZ):
    """AEV for one shard: coordinates (b,N,3), species (b,N) -> (b,N,384)."""
    b, n, _ = coordinates.shape
    valid = species >= 0
    sp = jnp.clip(species, 0, NUM_SPECIES - 1)
    vec = coordinates[:, None, :, :] - coordinates[:, :, None, :]  # (b,N,N,3)
    d2 = jnp.sum(vec * vec, axis=-1)
    eye = jnp.eye(n, dtype=bool)
    d = jnp.sqrt(jnp.where(d2 > 0, d2, 1.0))
    pair_ok = valid[:, :, None] & valid[:, None, :] & (~eye)[None]

    # radial
    mR = pair_ok & (d <= RCR)
    fcR = 0.5 * jnp.cos(d * (np.pi / RCR)) + 0.5
    rt = 0.25 * jnp.exp(-EtaR[:, None] * (d[..., None, None] - ShfR[None, :]) ** 2) \
        * fcR[..., None, None]
    rt = rt * mR[..., None, None].astype(rt.dtype)
    rt = rt.reshape(b, n, n, -1)  # (b,N,N,16)
    oh = jax.nn.one_hot(species, NUM_SPECIES, dtype=rt.dtype)  # (b,N,4)
    radial = jnp.einsum('bijr,bjs->bisr', rt, oh).reshape(b, n, -1)  # (b,N,64)

    # angular — arccos-free: with x = 0.95*cos(theta) the reference's
    # cos(arccos(x) - ShfZ) equals x*cos(ShfZ) + sqrt(1-x^2)*sin(ShfZ)
    # (valid since arccos(x) is in [0, pi] where sin >= 0).
    mA = pair_ok & (d <= RCA)
    tmask = mA[:, :, :, None] & mA[:, :, None, :] & (~eye)[None, None, :, :]
    dij = d[:, :, :, None]
    dik = d[:, :, None, :]
    cosang = jnp.einsum('bijd,bikd->bijk', vec, vec) / jnp.maximum(dij * dik, 1e-10)
    x = 0.95 * cosang
    sx = jnp.sqrt(jnp.maximum(1.0 - x * x, 0.0))
    cosdelta = x[..., None] * jnp.cos(ShfZ) + sx[..., None] * jnp.sin(ShfZ)  # (b,N,N,N,8)
    base = (1.0 + cosdelta) * 0.5
    zeta = float(Zeta[0])
    if zeta == int(zeta) and 1 <= zeta <= 64:
        # integer power via square-and-multiply (avoids exp/log lowering)
        e = int(zeta)
        acc = None
        sq = base
        while e:
            if e & 1:
                acc = sq if acc is None else acc * sq
            e >>= 1
            if e:
                sq = sq * sq
        f1 = acc
    else:
        f1 = base ** zeta
    fcA = 0.5 * jnp.cos(d * (np.pi / RCA)) + 0.5
    fprod = fcA[:, :, :, None] * fcA[:, :, None, :]
    dmean = (dij + dik) * 0.5
    f2 = jnp.exp(-EtaA[0] * (dmean[..., None] - ShfA) ** 2)  # (b,N,N,N,4)
    w = 2.0 * fprod * tmask.astype(f1.dtype)  # (b,N,N,N)
    # term (b,N,N,N,4,8) -> flatten (ShfA, ShfZ) in that order = reference's
    # (EtaA, Zeta, ShfA, ShfZ) flattening with singleton EtaA/Zeta.
    term = w[..., None, None] * f2[..., :, None] * f1[..., None, :]
    term = term.reshape(b, n, n, n, 32)
    npairs = NUM_SPECIES * (NUM_SPECIES + 1) // 2
    pidx = jnp.asarray(_TRIU)[sp[:, :, None], sp[:, None, :]]  # (b,N,N)
    poh = jax.nn.one_hot(pidx, npairs, dtype=term.dtype)  # (b,N,N,10)
    # bf16 for the scatter contraction only: poh is exactly representable
    # (0/1) and term values are in [0,2], so error is ~1e-3 — well under
    # tolerance — while the PE runs at 2x rate.
    angular = 0.5 * jnp.einsum(
        'bijkf,bjkp->bipf',
        term.astype(jnp.bfloat16), poh.astype(jnp.bfloat16),
        preferred_element_type=jnp.float32)
    angular = angular.reshape(b, n, -1)  # (b,N,320)

    return jnp.concatenate([radial, angular], axis=-1)  # (b,N,384)


_COMPILED = {}


def _get_fn(EtaR, ShfR, EtaA, Zeta, ShfA, ShfZ):
    key = (float(EtaR[0]), tuple(np.asarray(ShfR).tolist()), float(EtaA[0]),
           float(Zeta[0]), tuple(np.asarray(ShfA).tolist()),
           tuple(np.asarray(ShfZ).tolist()))
    if key not in _COMPILED:
        cEtaR = np.asarray(EtaR, np.float32)
        cShfR = np.asarray(ShfR, np.float32)
        cEtaA = np.asarray(EtaA, np.float32)
        cZeta = np.asarray(Zeta, np.float32)
        cShfA = np.asarray(ShfA, np.float32)
        cShfZ = np.asarray(ShfZ, np.float32)

        def shard_fn(coords, spec):
            return _aev_shard(coords, spec, cEtaR, cShfR, cEtaA, cZeta,
                              cShfA, cShfZ)

        _COMPILED[key] = jax.pmap(shard_fn, devices=jax.devices()[:NCORES])
    return _COMPILED[key]


def kernel(coordinates, EtaR, ShfR, EtaA, Zeta, ShfA, ShfZ, species):
    coords = np.ascontiguousarray(np.asarray(coordinates, np.float32)) \
        .reshape(NCORES, B // NCORES, N, 3)
    spec = np.ascontiguousarray(np.asarray(species, np.int32)) \
        .reshape(NCORES, B // NCORES, N)
    fn = _get_fn(np.asarray(EtaR), np.asarray(ShfR), np.asarray(EtaA),
                 np.asarray(Zeta), np.asarray(ShfA), np.asarray(ShfZ))
    out = fn(coords, spec)  # (8, 16, N, 384)
    return np.asarray(out, np.float32).reshape(B, N, 384)


# revision 6
# speedup vs baseline: 4.2538x; 1.2302x over previous
"""AEVComputer (ANI-style atomic environment vectors) on 8 NeuronCores.

Pure data-parallel: the 128-molecule batch is sharded 16-per-core across the
8 trn2 NeuronCores; all small parameter tensors are replicated. Output is
gathered back to the full (128, 32, 384) array.

Self-contained: shapes/sharding hardcoded for B=128, N=32, 8 cores.
"""
import numpy as np
import jax
import jax.numpy as jnp

RCR = 5.2
RCA = 3.5
NUM_SPECIES = 4
B, N = 128, 32
NCORES = 8


def _triu_index(ns):
    i, j = np.triu_indices(ns)
    t = np.zeros((ns, ns), dtype=np.int32)
    idx = np.arange(len(i), dtype=np.int32)
    t[i, j] = idx
    t[j, i] = idx
    return t


_TRIU = _triu_index(NUM_SPECIES)


def _aev_shard(coordinates, species, EtaR, ShfR, EtaA, Zeta, ShfA, ShfZ):
    """AEV for one shard: coordinates (b,N,3), species (b,N) -> (b,N,384)."""
    b, n, _ = coordinates.shape
    valid = species >= 0
    sp = jnp.clip(species, 0, NUM_SPECIES - 1)
    vec = coordinates[:, None, :, :] - coordinates[:, :, None, :]  # (b,N,N,3)
    d2 = jnp.sum(vec * vec, axis=-1)
    eye = jnp.eye(n, dtype=bool)
    d = jnp.sqrt(jnp.where(d2 > 0, d2, 1.0))
    pair_ok = valid[:, :, None] & valid[:, None, :] & (~eye)[None]

    # radial
    mR = pair_ok & (d <= RCR)
    fcR = 0.5 * jnp.cos(d * (np.pi / RCR)) + 0.5
    rt = 0.25 * jnp.exp(-EtaR[:, None] * (d[..., None, None] - ShfR[None, :]) ** 2) \
        * fcR[..., None, None]
    rt = rt * mR[..., None, None].astype(rt.dtype)
    rt = rt.reshape(b, n, n, -1)  # (b,N,N,16)
    oh = jax.nn.one_hot(species, NUM_SPECIES, dtype=rt.dtype)  # (b,N,4)
    radial = jnp.einsum('bijr,bjs->bisr', rt, oh).reshape(b, n, -1)  # (b,N,64)

    # angular — arccos-free: with x = 0.95*cos(theta) the reference's
    # cos(arccos(x) - ShfZ) equals x*cos(ShfZ) + sqrt(1-x^2)*sin(ShfZ)
    # (valid since arccos(x) is in [0, pi] where sin >= 0).
    mA = pair_ok & (d <= RCA)
    tmask = mA[:, :, :, None] & mA[:, :, None, :] & (~eye)[None, None, :, :]
    dij = d[:, :, :, None]
    dik = d[:, :, None, :]
    cosang = jnp.einsum('bijd,bikd->bijk', vec, vec) / jnp.maximum(dij * dik, 1e-10)
    x = 0.95 * cosang
    sx = jnp.sqrt(jnp.maximum(1.0 - x * x, 0.0))
    cosdelta = x[..., None] * jnp.cos(ShfZ) + sx[..., None] * jnp.sin(ShfZ)  # (b,N,N,N,8)
    base = (1.0 + cosdelta) * 0.5
    zeta = float(Zeta[0])
    if zeta == int(zeta) and 1 <= zeta <= 64:
        # integer power via square-and-multiply (avoids exp/log lowering)
        e = int(zeta)
        acc = None
        sq = base
        while e:
            if e & 1:
                acc = sq if acc is None else acc * sq
            e >>= 1
            if e:
                sq = sq * sq
        f1 = acc
    else:
        f1 = base ** zeta
    fcA = 0.5 * jnp.cos(d * (np.pi / RCA)) + 0.5
    fprod = fcA[:, :, :, None] * fcA[:, :, None, :]
    dmean = (dij + dik) * 0.5
    f2 = jnp.exp(-EtaA[0] * (dmean[..., None] - ShfA) ** 2)  # (b,N,N,N,4)
    w = 2.0 * fprod * tmask.astype(f1.dtype)  # (b,N,N,N)
    # term (b,N,N,N,4,8) -> flatten (ShfA, ShfZ) in that order = reference's
    # (EtaA, Zeta, ShfA, ShfZ) flattening with singleton EtaA/Zeta.
    # Assemble in bf16: f1/f2/w are each accurate in fp32 before the cast,
    # values lie in [0,2], and the products feed a bf16 matmul anyway — the
    # DVE runs bf16 tensor_tensor at 2x rate.
    bf = jnp.bfloat16
    wf2 = w[..., None].astype(bf) * f2.astype(bf)  # (b,N,N,N,4)
    term = wf2[..., :, None] * f1.astype(bf)[..., None, :]
    term = term.reshape(b, n, n, n, 32)
    npairs = NUM_SPECIES * (NUM_SPECIES + 1) // 2
    pidx = jnp.asarray(_TRIU)[sp[:, :, None], sp[:, None, :]]  # (b,N,N)
    poh = jax.nn.one_hot(pidx, npairs, dtype=term.dtype)  # (b,N,N,10)
    # bf16 for the scatter contraction only: poh is exactly representable
    # (0/1) and term values are in [0,2], so error is ~1e-3 — well under
    # tolerance — while the PE runs at 2x rate.
    angular = 0.5 * jnp.einsum(
        'bijkf,bjkp->bipf',
        term, poh.astype(jnp.bfloat16),
        preferred_element_type=jnp.float32)
    angular = angular.reshape(b, n, -1)  # (b,N,320)

    return jnp.concatenate([radial, angular], axis=-1)  # (b,N,384)


_COMPILED = {}


def _get_fn(EtaR, ShfR, EtaA, Zeta, ShfA, ShfZ):
    key = (float(EtaR[0]), tuple(np.asarray(ShfR).tolist()), float(EtaA[0]),
           float(Zeta[0]), tuple(np.asarray(ShfA).tolist()),
           tuple(np.asarray(ShfZ).tolist()))
    if key not in _COMPILED:
        cEtaR = np.asarray(EtaR, np.float32)
        cShfR = np.asarray(ShfR, np.float32)
        cEtaA = np.asarray(EtaA, np.float32)
        cZeta = np.asarray(Zeta, np.float32)
        cShfA = np.asarray(ShfA, np.float32)
        cShfZ = np.asarray(ShfZ, np.float32)

        def shard_fn(coords, spec):
            return _aev_shard(coords, spec, cEtaR, cShfR, cEtaA, cZeta,
                              cShfA, cShfZ)

        _COMPILED[key] = jax.pmap(shard_fn, devices=jax.devices()[:NCORES])
    return _COMPILED[key]


def kernel(coordinates, EtaR, ShfR, EtaA, Zeta, ShfA, ShfZ, species):
    coords = np.ascontiguousarray(np.asarray(coordinates, np.float32)) \
        .reshape(NCORES, B // NCORES, N, 3)
    spec = np.ascontiguousarray(np.asarray(species, np.int32)) \
        .reshape(NCORES, B // NCORES, N)
    fn = _get_fn(np.asarray(EtaR), np.asarray(ShfR), np.asarray(EtaA),
                 np.asarray(Zeta), np.asarray(ShfA), np.asarray(ShfZ))
    out = fn(coords, spec)  # (8, 16, N, 384)
    return np.asarray(out, np.float32).reshape(B, N, 384)
